# revision 1
# baseline (speedup 1.0000x reference)
"""Trainium2 Bass kernel for cosine-similarity multi-head attention.

Math (per batch element b):
    context = query @ w_q.T + b_q                    # [S, 120]
    ctx     = context * weight_tensor                # bcast [1,120]
    ctx_n   = ctx / max(||ctx||_2(axis=-1), 1e-12)   # L2 normalize
    scores  = ctx_n @ ctx_n.T                        # [S, S]
    out     = softmax(where(mask==0, -1e9, scores))  # row softmax

Sharding: data-parallel over batch. 8 batch elements -> 8 NeuronCores,
each core computes its own [S, S] output tile independently.

Kernel structure per core:
  Phase 0: load + transpose w_q (fold weight_tensor/bias in), identity.
  Phase 1: per 128-row s-tile: transpose query via PE, project to
           context, L2-normalize rows (sqrt + Newton refine), transpose
           back into a persistent ctxT [120, S] SBUF tensor.
  Phase 2: per 128-row q-tile: PE matmul scores chunks -> ACT exp ->
           DVE tensor_tensor_reduce (mask multiply + row-sum fused) ->
           reciprocal -> scale -> DMA out.  Softmax skips the row-max
           subtraction: scores are cosine similarities in [-1, 1], and
           masked entries are exactly zeroed by the mask multiply.
"""

import sys

if "/opt/trn_rl_repo" not in sys.path:
    sys.path.insert(0, "/opt/trn_rl_repo")

from contextlib import ExitStack

import numpy as np

import concourse.bass as bass
import concourse.mybir as mybir
import concourse.tile as tile
from concourse import bacc
from concourse.dve_ops import TENSOR_TENSOR_REDUCE as TTR_OP
from concourse.masks import make_identity

D_MODEL = 512
H_DIM = 120
N_CORES = 8
P = 128  # partition tile

F32 = mybir.dt.float32
I32 = mybir.dt.int32
Alu = mybir.AluOpType
Act = mybir.ActivationFunctionType

CFG = dict(
    chunk=1024,      # phase-2 column chunk (multiple of 512)
    mask_bufs=9,     # deep prefetch; mask tile doubles as the output buffer
    ech_bufs=4,      # small exp-chunk temps [128, chunk]
    ps2_bufs=3,      # phase-2 psum tiles [128, chunk]
    dma_split=4096,  # column width per dma_start for mask/out
    scores_f32r=True,  # float32r (tf32-like) for the big SxS matmul
)


def build_nc(S: int = 4096):
    nc = bacc.Bacc("TRN2", target_bir_lowering=False, debug=False)

    q_dram = nc.dram_tensor("query", [S, D_MODEL], F32, kind="ExternalInput")
    m_dram = nc.dram_tensor("mask", [S, S], I32, kind="ExternalInput")
    wq_dram = nc.dram_tensor("w_q", [H_DIM, D_MODEL], F32, kind="ExternalInput")
    bq_dram = nc.dram_tensor("b_q", [H_DIM], F32, kind="ExternalInput")
    wt_dram = nc.dram_tensor("weight_tensor", [1, H_DIM], F32, kind="ExternalInput")
    out_dram = nc.dram_tensor("out", [S, S], F32, kind="ExternalOutput")

    NT = S // P                      # 128-row tiles
    CHUNK = min(CFG["chunk"], S)
    NCH = S // CHUNK
    DSP = min(CFG["dma_split"], S)   # dma column split
    NDS = S // DSP
    ND = D_MODEL // P                # 4 chunks of contraction dim

    with tile.TileContext(nc) as tc, ExitStack() as ctx:
        singles = ctx.enter_context(tc.tile_pool(name="singles", bufs=1))

        # ---------- Phase 0: constants ----------
        ident = singles.tile([P, P], F32)
        make_identity(nc, ident)

        # weight_tensor broadcast to all 128 partitions: [128, 120]
        wtb = singles.tile([P, H_DIM], F32)
        nc.gpsimd.dma_start(
            out=wtb,
            in_=bass.AP(tensor=wt_dram, offset=0, ap=[[0, P], [1, H_DIM]]),
        )

        # b_q * weight_tensor -> bw [1, 120]
        bq_sb = singles.tile([1, H_DIM], F32)
        nc.gpsimd.dma_start(
            out=bq_sb,
            in_=bass.AP(tensor=bq_dram, offset=0, ap=[[0, 1], [1, H_DIM]]),
        )
        bw = singles.tile([1, H_DIM], F32)
        nc.vector.tensor_mul(bw, bq_sb, wtb[:1, :])

        ones_row = singles.tile([1, P], F32)
        nc.vector.memset(ones_row, 1.0)

        # w_q [120, 512] -> transposed+scaled wqTs [4x128, 120]
        wq_sb = singles.tile([H_DIM, D_MODEL], F32)
        nc.sync.dma_start(out=wq_sb, in_=wq_dram.ap())
        wqTs = singles.tile([P, ND * H_DIM], F32)

        # persistent normalized-transposed context [120 (pad 128), S].
        # Stored as float32r so the SxS matmul runs at 1 cycle/row (4x fp32);
        # the eviction copies below perform the f32 -> f32r rounding.
        ctxT = singles.tile([P, S], mybir.dt.float32r if CFG["scores_f32r"]
                            else F32)

        with ExitStack() as ph0:
            ps_w = ph0.enter_context(
                tc.tile_pool(name="ps_w", bufs=2, space="PSUM"))
            for c in range(ND):
                wqT_ps = ps_w.tile([P, H_DIM], F32)
                nc.tensor.transpose(
                    wqT_ps, wq_sb[:, c * P:(c + 1) * P], ident[:H_DIM, :H_DIM])
                # evict + fold in weight_tensor scale
                nc.vector.tensor_mul(
                    wqTs[:, c * H_DIM:(c + 1) * H_DIM], wqT_ps, wtb)

        with ExitStack() as ph1:
            # ---------- Phase 1: build ctxT ----------
            qin_p = ph1.enter_context(tc.tile_pool(name="qin", bufs=4))
            qt_p = ph1.enter_context(tc.tile_pool(name="qt", bufs=2))
            tmp_p = ph1.enter_context(tc.tile_pool(name="ph1tmp", bufs=2))
            st_p = ph1.enter_context(tc.tile_pool(name="ph1st", bufs=2))
            ps_t = ph1.enter_context(
                tc.tile_pool(name="ps_t", bufs=3, space="PSUM"))
            ps_c = ph1.enter_context(
                tc.tile_pool(name="ps_c", bufs=2, space="PSUM"))
            ps_ct = ph1.enter_context(
                tc.tile_pool(name="ps_ct", bufs=2, space="PSUM"))

            for i in range(NT):
                s0 = i * P
                # query via SWDGE so the sync ring is free for mask prefetch
                q_in = qin_p.tile([P, D_MODEL], F32)
                nc.gpsimd.dma_start(out=q_in, in_=q_dram[s0:s0 + P, :])

                # transpose query tile -> qT [d, s] chunks
                qT = qt_p.tile([P, D_MODEL], F32)
                for c in range(ND):
                    tp = ps_t.tile([P, P], F32, tag="tp")
                    nc.tensor.transpose(tp, q_in[:, c * P:(c + 1) * P], ident)
                    eng = nc.vector if c % 2 == 0 else nc.scalar
                    if eng is nc.vector:
                        nc.vector.tensor_copy(qT[:, c * P:(c + 1) * P], tp)
                    else:
                        nc.scalar.copy(qT[:, c * P:(c + 1) * P], tp)

                # context tile [s=128, k=120] = q @ (w_q * wt).T + b*wt
                ctx_ps = ps_c.tile([P, H_DIM], F32)
                for c in range(ND):
                    nc.tensor.matmul(
                        ctx_ps,
                        lhsT=qT[:, c * P:(c + 1) * P],
                        rhs=wqTs[:, c * H_DIM:(c + 1) * H_DIM],
                        start=(c == 0), stop=False)
                nc.tensor.matmul(
                    ctx_ps, lhsT=ones_row, rhs=bw, start=False, stop=True)

                # row L2 norm^2: ACT Square with free-dim accumulate
                sq = tmp_p.tile([P, H_DIM], F32, tag="sq")
                nsq = st_p.tile([P, 1], F32, tag="nsq")
                nc.scalar.activation(sq, ctx_ps, Act.Square, accum_out=nsq)

                # norm = sqrt(nsq), one Newton step, then rstd = 2/(s0+x/s0)
                sroot = st_p.tile([P, 1], F32, tag="sroot")
                nc.scalar.activation(sroot, nsq, Act.Sqrt)
                r0 = st_p.tile([P, 1], F32, tag="r0")
                nc.vector.reciprocal(r0, sroot)
                t1 = st_p.tile([P, 1], F32, tag="t1")
                nc.vector.tensor_mul(t1, nsq, r0)
                ssum = st_p.tile([P, 1], F32, tag="ssum")
                nc.vector.tensor_add(ssum, sroot, t1)
                nc.vector.tensor_scalar_max(ssum, ssum, 2e-12)
                u = st_p.tile([P, 1], F32, tag="u")
                nc.vector.reciprocal(u, ssum)
                rstd = st_p.tile([P, 1], F32, tag="rstd")
                nc.vector.tensor_scalar_mul(rstd, u, 2.0)

                # normalize + evict: ctx_n [s, k]
                ctxn = tmp_p.tile([P, H_DIM], F32, tag="ctxn")
                nc.scalar.activation(ctxn, ctx_ps, Act.Copy, scale=rstd)

                # transpose to [k, s] and park into ctxT
                ctxT_ps = ps_ct.tile([H_DIM, P], F32)
                nc.tensor.transpose(ctxT_ps, ctxn, ident)
                if i % 2 == 0:
                    nc.vector.tensor_copy(ctxT[:H_DIM, s0:s0 + P], ctxT_ps)
                else:
                    nc.scalar.copy(ctxT[:H_DIM, s0:s0 + P], ctxT_ps)

        # ---------- Phase 2: scores + masked softmax ----------
        with ExitStack() as ph2:
            mask_p = ph2.enter_context(
                tc.tile_pool(name="maskp", bufs=CFG["mask_bufs"]))
            ech_p = ph2.enter_context(
                tc.tile_pool(name="echp", bufs=CFG["ech_bufs"]))
            sum_p = ph2.enter_context(tc.tile_pool(name="sump", bufs=3))
            ps2 = ph2.enter_context(
                tc.tile_pool(name="ps2", bufs=CFG["ps2_bufs"], space="PSUM"))

            for i in range(NT):
                q0 = i * P
                mask_sb = mask_p.tile([P, S], I32)
                for d in range(NDS):
                    nc.sync.dma_start(
                        out=mask_sb[:, d * DSP:(d + 1) * DSP],
                        in_=m_dram[q0:q0 + P, d * DSP:(d + 1) * DSP])
                # f32 view of the same bytes: masked exp overwrites the mask
                # tile in place, so one 16KB/partition pool serves mask in,
                # softmax scratch, and the store buffer.
                maskf = mask_sb.bitcast(F32)

                sums = sum_p.tile([P, NCH], F32, tag="sums")
                lhsT = ctxT[:H_DIM, q0:q0 + P]
                for j in range(NCH):
                    c0 = j * CHUNK
                    sc_ps = ps2.tile([P, CHUNK], F32)
                    for h in range(CHUNK // 512):
                        nc.tensor.matmul(
                            sc_ps[:, h * 512:(h + 1) * 512],
                            lhsT=lhsT,
                            rhs=ctxT[:H_DIM, c0 + h * 512:c0 + (h + 1) * 512],
                            start=True, stop=True)
                    # exp (scores in [-1, 1]; masked entries zeroed next)
                    ech = ech_p.tile([P, CHUNK], F32)
                    nc.scalar.activation(ech, sc_ps, Act.Exp)
                    # fused mask-multiply + row-sum (chained across chunks);
                    # custom-DVE uop: out = in0*in1*s1, accum = s0 + sum(out)
                    nc.vector._custom_dve(
                        TTR_OP,
                        out=maskf[:, c0:c0 + CHUNK],
                        in0=ech,
                        in1=mask_sb[:, c0:c0 + CHUNK],
                        s0=(0.0 if j == 0 else sums[:, j - 1:j]),
                        s1=1.0,
                        accum_out=sums[:, j:j + 1])

                rden = sum_p.tile([P, 1], F32, tag="rden")
                nc.vector.reciprocal(rden, sums[:, NCH - 1:NCH])

                # normalize in place, alternating engines, then store via the
                # ACT HWDGE ring (so blocked stores can't head-of-line-block
                # mask prefetch on the sync ring)
                for j in range(NCH):
                    c0 = j * CHUNK
                    if j % 2 == 0:
                        nc.scalar.activation(
                            maskf[:, c0:c0 + CHUNK], maskf[:, c0:c0 + CHUNK],
                            Act.Copy, scale=rden)
                    else:
                        nc.vector.tensor_scalar_mul(
                            maskf[:, c0:c0 + CHUNK], maskf[:, c0:c0 + CHUNK],
                            rden)
                for d in range(NDS):
                    nc.scalar.dma_start(
                        out=out_dram[q0:q0 + P, d * DSP:(d + 1) * DSP],
                        in_=maskf[:, d * DSP:(d + 1) * DSP])

    nc.compile()
    return nc


def _run(nc, in_maps, trace=False, tmpdir=None):
    from concourse import bass_utils
    return bass_utils.run_bass_kernel_spmd(
        nc, in_maps, core_ids=list(range(len(in_maps))), trace=trace,
        tmpdir=tmpdir)


def kernel(**inputs: np.ndarray) -> np.ndarray:
    query = np.ascontiguousarray(np.asarray(inputs["query"], np.float32))
    mask = np.ascontiguousarray(np.asarray(inputs["mask"], np.int32))
    w_q = np.ascontiguousarray(np.asarray(inputs["w_q"], np.float32))
    b_q = np.ascontiguousarray(np.asarray(inputs["b_q"], np.float32))
    wt = np.ascontiguousarray(
        np.asarray(inputs["weight_tensor"], np.float32).reshape(1, H_DIM))

    B, S, _ = query.shape
    assert B == N_CORES
    nc = build_nc(S)
    in_maps = [
        dict(query=query[b], mask=mask[b], w_q=w_q, b_q=b_q, weight_tensor=wt)
        for b in range(B)
    ]
    res = _run(nc, in_maps)
    return np.stack([res.results[b]["out"] for b in range(B)], axis=0)



# revision 2
# speedup vs baseline: 1.6210x; 1.6210x over previous
"""Trainium2 Bass kernel for cosine-similarity multi-head attention.

Math (per batch element b):
    context = query @ w_q.T + b_q                    # [S, 120]
    ctx     = context * weight_tensor                # bcast [1,120]
    ctx_n   = ctx / max(||ctx||_2(axis=-1), 1e-12)   # L2 normalize
    scores  = ctx_n @ ctx_n.T                        # [S, S]
    out     = softmax(where(mask==0, -1e9, scores))  # row softmax

Sharding: data-parallel over batch. 8 batch elements -> 8 NeuronCores.

Key ideas vs the v1 kernel (436us):
  * Host-side input compression: the mask is converted to an ADDITIVE
    fp8 bias {-64 (masked), 0 (keep)} -> mask DMA drops 64 MiB -> 16 MiB.
    The query is transposed on host so phase 1 needs no PE transposes.
  * The mask bias is injected into the scores PSUM by the PE itself
    (identity-weights matmul with the fp8 mask as the streaming operand),
    so the DVE never touches the mask and the slow 1x-mode
    tensor_tensor op disappears. exp(score-64) == 0 in fp16.
  * exp runs on ACT (PSUM -> SBUF fp16) with accum_out producing the
    row sums for free; one 4x-mode DVE tensor_scalar normalizes in
    place; the output is stored as fp16 (32 MiB instead of 64) and
    upcast on host.
Engine budget per core: DMA ~56 MiB, ACT ~150us, PE ~150us, DVE ~70us.
"""

import sys

if "/opt/trn_rl_repo" not in sys.path:
    sys.path.insert(0, "/opt/trn_rl_repo")

from contextlib import ExitStack

import numpy as np

import concourse.bass as bass
import concourse.mybir as mybir
import concourse.tile as tile
from concourse import bacc
from concourse.masks import make_identity

D_MODEL = 512
H_DIM = 120
N_CORES = 8
P = 128
MASK_BIAS = -64.0  # additive mask value; exp(score + MASK_BIAS) == 0 in fp16

F32 = mybir.dt.float32
F32R = mybir.dt.float32r
F16 = mybir.dt.float16
F8 = mybir.dt.float8e4
Alu = mybir.AluOpType
Act = mybir.ActivationFunctionType

CFG = dict(
    mask_bufs=10,    # fp8 mask tiles [128, S] (4KB/partition each)
    ech_bufs=3,      # fp16 exp/out tiles [128, S] (8KB/partition each)
    qt_bufs=4,       # phase-1 transposed-query tiles
    newton=True,     # refine ACT sqrt with one Newton step (cheap, phase 1)
)


def build_nc(S: int = 4096):
    nc = bacc.Bacc("TRN2", target_bir_lowering=False, debug=False)

    qT_dram = nc.dram_tensor("queryT", [D_MODEL, S], F32, kind="ExternalInput")
    m_dram = nc.dram_tensor("maskb", [S, S], F8, kind="ExternalInput")
    wq_dram = nc.dram_tensor("w_q", [H_DIM, D_MODEL], F32, kind="ExternalInput")
    bq_dram = nc.dram_tensor("b_q", [H_DIM], F32, kind="ExternalInput")
    wt_dram = nc.dram_tensor("weight_tensor", [1, H_DIM], F32, kind="ExternalInput")
    out_dram = nc.dram_tensor("out", [S, S], F16, kind="ExternalOutput")

    NT = S // P          # 128-row tiles
    ND = D_MODEL // P    # 4 contraction chunks
    HCH = 2048           # phase-2 half-chunk (4 PSUM banks)
    NH = S // HCH

    with tile.TileContext(nc) as tc, ExitStack() as ctx:
        singles = ctx.enter_context(tc.tile_pool(name="singles", bufs=1))

        # ---------- Phase 0: constants ----------
        ident = singles.tile([P, P], F32)
        make_identity(nc, ident)
        # fp8 identity: stationary weights for the PE mask injection
        ident8 = singles.tile([P, P], F8)
        nc.vector.tensor_copy(ident8, ident)

        wtb = singles.tile([P, H_DIM], F32)
        nc.gpsimd.dma_start(
            out=wtb,
            in_=bass.AP(tensor=wt_dram, offset=0, ap=[[0, P], [1, H_DIM]]),
        )
        bq_sb = singles.tile([1, H_DIM], F32)
        nc.gpsimd.dma_start(
            out=bq_sb,
            in_=bass.AP(tensor=bq_dram, offset=0, ap=[[0, 1], [1, H_DIM]]),
        )
        bw = singles.tile([1, H_DIM], F32)
        nc.vector.tensor_mul(bw, bq_sb, wtb[:1, :])

        ones_row = singles.tile([1, P], F32)
        nc.vector.memset(ones_row, 1.0)

        # w_q [120, 512] -> transposed+scaled wqTs [4x128, 120]
        wq_sb = singles.tile([H_DIM, D_MODEL], F32)
        nc.sync.dma_start(out=wq_sb, in_=wq_dram.ap())
        wqTs = singles.tile([P, ND * H_DIM], F32)
        with ExitStack() as ph0:
            ps_w = ph0.enter_context(
                tc.tile_pool(name="ps_w", bufs=2, space="PSUM"))
            for c in range(ND):
                wqT_ps = ps_w.tile([P, H_DIM], F32)
                nc.tensor.transpose(
                    wqT_ps, wq_sb[:, c * P:(c + 1) * P], ident[:H_DIM, :H_DIM])
                nc.vector.tensor_mul(
                    wqTs[:, c * H_DIM:(c + 1) * H_DIM], wqT_ps, wtb)

        # persistent normalized-transposed context [120 (pad 128), S], f32r
        ctxT = singles.tile([P, S], F32R)

        # ---------- Phase 1: build ctxT ----------
        with ExitStack() as ph1:
            qt_p = ph1.enter_context(tc.tile_pool(name="qt", bufs=CFG["qt_bufs"]))
            sq_p = ph1.enter_context(tc.tile_pool(name="sq", bufs=2))
            st_p = ph1.enter_context(tc.tile_pool(name="ph1st", bufs=2))
            cn_p = ph1.enter_context(tc.tile_pool(name="ctxn", bufs=2))
            ps_c = ph1.enter_context(
                tc.tile_pool(name="ps_c", bufs=2, space="PSUM"))
            ps_ct = ph1.enter_context(
                tc.tile_pool(name="ps_ct", bufs=2, space="PSUM"))

            for i in range(NT):
                s0 = i * P
                # transposed query tile [d=128 (per chunk), 4 chunks x 128 s]
                qt = qt_p.tile([P, D_MODEL], F32)
                nc.gpsimd.dma_start(
                    out=qt,
                    in_=bass.AP(tensor=qT_dram, offset=s0,
                                ap=[[S, P], [P * S, ND], [1, P]]),
                )

                # ctx tile [s=128, k=120] = q @ (w_q * wt).T + b*wt
                ctx_ps = ps_c.tile([P, H_DIM], F32)
                for c in range(ND):
                    nc.tensor.matmul(
                        ctx_ps,
                        lhsT=qt[:, c * P:(c + 1) * P],
                        rhs=wqTs[:, c * H_DIM:(c + 1) * H_DIM],
                        start=(c == 0), stop=False)
                nc.tensor.matmul(
                    ctx_ps, lhsT=ones_row, rhs=bw, start=False, stop=True)

                # row L2 norm^2 via ACT Square with free-dim accumulate
                sq = sq_p.tile([P, H_DIM], F32, tag="sq")
                nsq = st_p.tile([P, 1], F32, tag="nsq")
                nc.scalar.activation(sq, ctx_ps, Act.Square, accum_out=nsq)

                sroot = st_p.tile([P, 1], F32, tag="sroot")
                nc.scalar.activation(sroot, nsq, Act.Sqrt)
                if CFG["newton"]:
                    # rstd = 2 / (s0 + nsq/s0): one Newton step for 1/sqrt
                    r0 = st_p.tile([P, 1], F32, tag="r0")
                    nc.vector.reciprocal(r0, sroot)
                    t1 = st_p.tile([P, 1], F32, tag="t1")
                    nc.vector.tensor_mul(t1, nsq, r0)
                    ssum = st_p.tile([P, 1], F32, tag="ssum")
                    nc.vector.tensor_add(ssum, sroot, t1)
                    rstd = st_p.tile([P, 1], F32, tag="rstd")
                    nc.vector.tensor_scalar(
                        rstd, ssum, 0.5, None, Alu.mult)
                    u = st_p.tile([P, 1], F32, tag="u")
                    nc.vector.reciprocal(u, rstd)
                    rstd = u
                else:
                    rstd = st_p.tile([P, 1], F32, tag="rstd")
                    nc.vector.reciprocal(rstd, sroot)

                # normalize + evict + transpose into ctxT
                ctxn = cn_p.tile([P, H_DIM], F32, tag="ctxn")
                nc.vector.tensor_scalar_mul(ctxn, ctx_ps, rstd)
                ctxT_ps = ps_ct.tile([H_DIM, P], F32)
                nc.tensor.transpose(ctxT_ps, ctxn, ident)
                if i % 2 == 0:
                    nc.vector.tensor_copy(ctxT[:H_DIM, s0:s0 + P], ctxT_ps)
                else:
                    nc.scalar.copy(ctxT[:H_DIM, s0:s0 + P], ctxT_ps)

        # ---------- Phase 2: scores + masked softmax ----------
        with ExitStack() as ph2:
            mask_p = ph2.enter_context(
                tc.tile_pool(name="maskp", bufs=CFG["mask_bufs"]))
            ech_p = ph2.enter_context(
                tc.tile_pool(name="echp", bufs=CFG["ech_bufs"]))
            sum_p = ph2.enter_context(tc.tile_pool(name="sump", bufs=3))
            ps2 = ph2.enter_context(
                tc.tile_pool(name="ps2", bufs=2, space="PSUM"))

            for i in range(NT):
                q0 = i * P
                mask_sb = mask_p.tile([P, S], F8)
                nc.sync.dma_start(out=mask_sb, in_=m_dram[q0:q0 + P, :])

                ech = ech_p.tile([P, S], F16)
                sums = sum_p.tile([P, NH], F32, tag="sums")
                lhsT = ctxT[:H_DIM, q0:q0 + P]
                for h in range(NH):
                    c0 = h * HCH
                    sc = ps2.tile([P, HCH], F32)
                    # PE injects the additive fp8 mask (identity weights),
                    # then accumulates the cosine scores on top.
                    for j in range(HCH // 512):
                        nc.tensor.matmul(
                            sc[:, j * 512:(j + 1) * 512],
                            lhsT=ident8,
                            rhs=mask_sb[:, c0 + j * 512:c0 + (j + 1) * 512],
                            start=True, stop=False)
                    for j in range(HCH // 512):
                        nc.tensor.matmul(
                            sc[:, j * 512:(j + 1) * 512],
                            lhsT=lhsT,
                            rhs=ctxT[:H_DIM, c0 + j * 512:c0 + (j + 1) * 512],
                            start=False, stop=True)
                    # exp -> fp16 (masked entries flush to exactly 0);
                    # accum_out gives the fp32 row sum for free
                    nc.scalar.activation(
                        ech[:, c0:c0 + HCH], sc, Act.Exp,
                        accum_out=sums[:, h:h + 1])

                s2 = sum_p.tile([P, 1], F32, tag="s2")
                nc.vector.tensor_add(s2, sums[:, 0:1], sums[:, 1:2])
                rden = sum_p.tile([P, 1], F32, tag="rden")
                nc.vector.reciprocal(rden, s2)

                # normalize in place (fp16 4x mode), store via gpsimd SWDGE
                for h in range(NH):
                    c0 = h * HCH
                    nc.vector.tensor_scalar_mul(
                        ech[:, c0:c0 + HCH], ech[:, c0:c0 + HCH], rden)
                nc.gpsimd.dma_start(out=out_dram[q0:q0 + P, :], in_=ech)

    nc.compile()
    return nc


def _run(nc, in_maps, trace=False, tmpdir=None):
    from concourse import bass_utils
    return bass_utils.run_bass_kernel_spmd(
        nc, in_maps, core_ids=list(range(len(in_maps))), trace=trace,
        tmpdir=tmpdir)


def prep_inputs(inputs):
    """Host-side shard + compress: returns per-core in_maps."""
    import ml_dtypes
    query = np.asarray(inputs["query"], np.float32)
    mask = np.asarray(inputs["mask"])
    w_q = np.ascontiguousarray(np.asarray(inputs["w_q"], np.float32))
    b_q = np.ascontiguousarray(np.asarray(inputs["b_q"], np.float32))
    wt = np.ascontiguousarray(
        np.asarray(inputs["weight_tensor"], np.float32).reshape(1, H_DIM))

    B, S, _ = query.shape
    f8 = ml_dtypes.float8_e4m3
    in_maps = []
    for b in range(B):
        qT = np.ascontiguousarray(query[b].T)
        # additive fp8 mask: 0 where kept, MASK_BIAS where masked out
        mb = np.where(mask[b] == 0, np.float32(MASK_BIAS),
                      np.float32(0.0)).astype(f8)
        in_maps.append(dict(queryT=qT, maskb=mb, w_q=w_q, b_q=b_q,
                            weight_tensor=wt))
    return in_maps, B, S


def kernel(**inputs: np.ndarray) -> np.ndarray:
    in_maps, B, S = prep_inputs(inputs)
    assert B == N_CORES
    nc = build_nc(S)
    res = _run(nc, in_maps)
    return np.stack(
        [res.results[b]["out"].astype(np.float32) for b in range(B)], axis=0)


# revision 3
# speedup vs baseline: 2.0739x; 1.2794x over previous
"""Trainium2 Bass kernel for cosine-similarity multi-head attention.

Math (per batch element b):
    context = query @ w_q.T + b_q                    # [S, 120]
    ctx     = context * weight_tensor                # bcast [1,120]
    ctx_n   = ctx / max(||ctx||_2(axis=-1), 1e-12)   # L2 normalize
    scores  = ctx_n @ ctx_n.T                        # [S, S]
    out     = softmax(where(mask==0, -1e9, scores))  # row softmax

Sharding: data-parallel over batch. 8 batch elements -> 8 NeuronCores.

Structure (v3):
  * Host compresses inputs: mask -> additive fp8 bias {-64, 0} (16 MiB),
    query -> pre-transposed fp16 (4 MiB). Output is fp16 (32 MiB),
    upcast on host. Total DMA ~52 MiB/core vs 136 MiB for the naive
    fp32/int32 layout.
  * Phase 1 builds a normalized-transposed fp16 context ctxT [120, S]:
    ctx tile via PE (fp16 weights -> fast FWL weight loads), row norms
    via ACT Square+Sqrt + DVE reciprocal, scale on DVE, PE transpose.
  * Phase 2 per 128-row tile: PE injects the additive fp8 mask into
    PSUM (identity-weights matmul) then accumulates fp16 cosine scores
    on top; ACT exp (PSUM -> fp16 SBUF) with accum_out row sums; DVE
    reciprocal + in-place 4x-mode normalize; gpsimd SWDGE stores.
  * A 16-matmul warmup burst at the end of phase 1 keeps the PE HAM
    clock-gate at 2.4 GHz going into phase 2.
"""

import sys

if "/opt/trn_rl_repo" not in sys.path:
    sys.path.insert(0, "/opt/trn_rl_repo")

from contextlib import ExitStack

import numpy as np

import concourse.bass as bass
import concourse.mybir as mybir
import concourse.tile as tile
from concourse import bacc
from concourse.masks import make_identity

D_MODEL = 512
H_DIM = 120
N_CORES = 8
P = 128
MASK_BIAS = -64.0  # additive mask value; exp(score + MASK_BIAS) == 0 in fp16

F32 = mybir.dt.float32
F16 = mybir.dt.float16
F8 = mybir.dt.float8e4
Alu = mybir.AluOpType
Act = mybir.ActivationFunctionType

CFG = dict(
    mask_bufs=12,    # fp8 mask tiles [128, S] (4KB/partition each)
    ech_bufs=6,      # fp16 exp/out tiles [128, S] (8KB/partition each)
    qt_bufs=6,       # phase-1 transposed-query tiles (fp16, 1KB each)
    sum_bufs=6,
    ph1_depth=4,     # phase-1 pipeline depth (stage pools)
    newton=False,    # refine ACT sqrt with one Newton step
    warmup_mms=16,   # PE warmup burst at phase-1 end
)


def build_nc(S: int = 4096):
    nc = bacc.Bacc("TRN2", target_bir_lowering=False, debug=False)

    qT_dram = nc.dram_tensor("queryT", [D_MODEL, S], F16, kind="ExternalInput")
    m_dram = nc.dram_tensor("maskb", [S, S], F8, kind="ExternalInput")
    wq_dram = nc.dram_tensor("w_q", [H_DIM, D_MODEL], F32, kind="ExternalInput")
    bq_dram = nc.dram_tensor("b_q", [H_DIM], F32, kind="ExternalInput")
    wt_dram = nc.dram_tensor("weight_tensor", [1, H_DIM], F32, kind="ExternalInput")
    out_dram = nc.dram_tensor("out", [S, S], F16, kind="ExternalOutput")

    NT = S // P          # 128-row tiles
    ND = D_MODEL // P    # 4 contraction chunks
    HCH = 2048           # phase-2 half-chunk (4 PSUM banks)
    NH = S // HCH

    with tile.TileContext(nc) as tc, ExitStack() as ctx:
        singles = ctx.enter_context(tc.tile_pool(name="singles", bufs=1))

        # ---------- Phase 0: constants ----------
        ident = singles.tile([P, P], F32)
        make_identity(nc, ident)
        # fp8 identity: stationary weights for the PE mask injection
        ident8 = singles.tile([P, P], F8)
        nc.vector.tensor_copy(ident8, ident)
        ident16 = singles.tile([P, P], F16)
        nc.vector.tensor_copy(ident16, ident)

        wtb = singles.tile([P, H_DIM], F32)
        nc.gpsimd.dma_start(
            out=wtb,
            in_=bass.AP(tensor=wt_dram, offset=0, ap=[[0, P], [1, H_DIM]]),
        )
        bq_sb = singles.tile([1, H_DIM], F32)
        nc.gpsimd.dma_start(
            out=bq_sb,
            in_=bass.AP(tensor=bq_dram, offset=0, ap=[[0, 1], [1, H_DIM]]),
        )
        bw = singles.tile([1, H_DIM], F32)
        nc.vector.tensor_mul(bw, bq_sb, wtb[:1, :])

        ones_row = singles.tile([1, P], F32)
        nc.vector.memset(ones_row, 1.0)

        # w_q [120, 512] -> transposed+scaled fp16 wqTs [4x128, 120]
        wq_sb = singles.tile([H_DIM, D_MODEL], F32)
        nc.sync.dma_start(out=wq_sb, in_=wq_dram.ap())
        wqTs = singles.tile([P, ND * H_DIM], F16)
        with ExitStack() as ph0:
            ps_w = ph0.enter_context(
                tc.tile_pool(name="ps_w", bufs=2, space="PSUM"))
            for c in range(ND):
                wqT_ps = ps_w.tile([P, H_DIM], F32)
                nc.tensor.transpose(
                    wqT_ps, wq_sb[:, c * P:(c + 1) * P], ident[:H_DIM, :H_DIM])
                nc.vector.tensor_mul(
                    wqTs[:, c * H_DIM:(c + 1) * H_DIM], wqT_ps, wtb)

        # persistent normalized-transposed context [120 (pad 128), S], fp16
        ctxT = singles.tile([P, S], F16)

        # ---------- Phase 1: build ctxT ----------
        D1 = CFG["ph1_depth"]
        with ExitStack() as ph1:
            qt_p = ph1.enter_context(tc.tile_pool(name="qt", bufs=CFG["qt_bufs"]))
            sq_p = ph1.enter_context(tc.tile_pool(name="sq", bufs=D1))
            st_p = ph1.enter_context(tc.tile_pool(name="ph1st", bufs=D1))
            cn_p = ph1.enter_context(tc.tile_pool(name="ctxn", bufs=D1))
            ps_c = ph1.enter_context(
                tc.tile_pool(name="ps_c", bufs=D1, space="PSUM"))
            ps_ct = ph1.enter_context(
                tc.tile_pool(name="ps_ct", bufs=3, space="PSUM"))
            ps_warm = ph1.enter_context(
                tc.tile_pool(name="ps_warm", bufs=1, space="PSUM"))

            for i in range(NT):
                s0 = i * P
                # transposed query tile [d=128 (per chunk), 4 chunks x 128 s]
                qt = qt_p.tile([P, D_MODEL], F16)
                nc.gpsimd.dma_start(
                    out=qt,
                    in_=bass.AP(tensor=qT_dram, offset=s0,
                                ap=[[S, P], [P * S, ND], [1, P]]),
                )

                # ctx tile [s=128, k=120] = q @ (w_q * wt).T + b*wt
                ctx_ps = ps_c.tile([P, H_DIM], F32)
                for c in range(ND):
                    nc.tensor.matmul(
                        ctx_ps,
                        lhsT=qt[:, c * P:(c + 1) * P],
                        rhs=wqTs[:, c * H_DIM:(c + 1) * H_DIM],
                        start=(c == 0), stop=False)
                nc.tensor.matmul(
                    ctx_ps, lhsT=ones_row, rhs=bw, start=False, stop=True)

                # row L2 norm^2 via ACT Square with free-dim accumulate
                sq = sq_p.tile([P, H_DIM], F32, tag="sq")
                nsq = st_p.tile([P, 1], F32, tag="nsq")
                nc.scalar.activation(sq, ctx_ps, Act.Square, accum_out=nsq)

                sroot = st_p.tile([P, 1], F32, tag="sroot")
                nc.scalar.activation(sroot, nsq, Act.Sqrt)
                if CFG["newton"]:
                    # rstd = 2 / (s0 + nsq/s0): one Newton step for 1/sqrt
                    r0 = st_p.tile([P, 1], F32, tag="r0")
                    nc.vector.reciprocal(r0, sroot)
                    t1 = st_p.tile([P, 1], F32, tag="t1")
                    nc.vector.tensor_mul(t1, nsq, r0)
                    ssum = st_p.tile([P, 1], F32, tag="ssum")
                    nc.vector.tensor_add(ssum, sroot, t1)
                    half = st_p.tile([P, 1], F32, tag="half")
                    nc.vector.tensor_scalar(half, ssum, 0.5, None, Alu.mult)
                    rstd = st_p.tile([P, 1], F32, tag="rstd")
                    nc.vector.reciprocal(rstd, half)
                else:
                    rstd = st_p.tile([P, 1], F32, tag="rstd")
                    nc.vector.reciprocal(rstd, sroot)

                # normalize (fp16) + transpose into ctxT
                ctxn = cn_p.tile([P, H_DIM], F16, tag="ctxn")
                nc.vector.tensor_scalar_mul(ctxn, ctx_ps, rstd)
                ctxT_ps = ps_ct.tile([H_DIM, P], F16)
                nc.tensor.transpose(ctxT_ps, ctxn, ident16)
                if i % 2 == 0:
                    nc.vector.tensor_copy(ctxT[:H_DIM, s0:s0 + P], ctxT_ps)
                else:
                    nc.scalar.copy(ctxT[:H_DIM, s0:s0 + P], ctxT_ps)

            # PE warmup burst: dense matmuls so the HAM clock-gate sits at
            # 2.4 GHz when phase 2 starts. Depends only on ctxT[:, :512].
            warm = ps_warm.tile([P, 512], F32)
            for w in range(CFG["warmup_mms"]):
                nc.tensor.matmul(
                    warm, lhsT=ctxT[:H_DIM, 0:P], rhs=ctxT[:H_DIM, 0:512],
                    start=True, stop=True)

        # ---------- Phase 2: scores + masked softmax ----------
        with ExitStack() as ph2:
            mask_p = ph2.enter_context(
                tc.tile_pool(name="maskp", bufs=CFG["mask_bufs"]))
            ech_p = ph2.enter_context(
                tc.tile_pool(name="echp", bufs=CFG["ech_bufs"]))
            sum_p = ph2.enter_context(
                tc.tile_pool(name="sump", bufs=CFG["sum_bufs"]))
            ps2 = ph2.enter_context(
                tc.tile_pool(name="ps2", bufs=2, space="PSUM"))

            for i in range(NT):
                q0 = i * P
                mask_sb = mask_p.tile([P, S], F8)
                nc.sync.dma_start(out=mask_sb, in_=m_dram[q0:q0 + P, :])

                ech = ech_p.tile([P, S], F16)
                sums = sum_p.tile([P, NH], F32, tag="sums")
                lhsT = ctxT[:H_DIM, q0:q0 + P]
                for h in range(NH):
                    c0 = h * HCH
                    sc = ps2.tile([P, HCH], F32)
                    # PE injects the additive fp8 mask (identity weights),
                    # then accumulates the cosine scores on top.
                    for j in range(HCH // 512):
                        nc.tensor.matmul(
                            sc[:, j * 512:(j + 1) * 512],
                            lhsT=ident8,
                            rhs=mask_sb[:, c0 + j * 512:c0 + (j + 1) * 512],
                            start=True, stop=False)
                    for j in range(HCH // 512):
                        nc.tensor.matmul(
                            sc[:, j * 512:(j + 1) * 512],
                            lhsT=lhsT,
                            rhs=ctxT[:H_DIM, c0 + j * 512:c0 + (j + 1) * 512],
                            start=False, stop=True)
                    # exp -> fp16 (masked entries flush to exactly 0);
                    # accum_out gives the fp32 row sum for free
                    nc.scalar.activation(
                        ech[:, c0:c0 + HCH], sc, Act.Exp,
                        accum_out=sums[:, h:h + 1])

                s2 = sum_p.tile([P, 1], F32, tag="s2")
                nc.vector.tensor_add(s2, sums[:, 0:1], sums[:, 1:2])
                rden = sum_p.tile([P, 1], F32, tag="rden")
                nc.vector.reciprocal(rden, s2)

                # normalize in place (fp16 4x mode), store via gpsimd SWDGE
                for h in range(NH):
                    c0 = h * HCH
                    nc.vector.tensor_scalar_mul(
                        ech[:, c0:c0 + HCH], ech[:, c0:c0 + HCH], rden)
                nc.gpsimd.dma_start(out=out_dram[q0:q0 + P, :], in_=ech)

    nc.compile()
    return nc


def _run(nc, in_maps, trace=False, tmpdir=None):
    from concourse import bass_utils
    return bass_utils.run_bass_kernel_spmd(
        nc, in_maps, core_ids=list(range(len(in_maps))), trace=trace,
        tmpdir=tmpdir)


def prep_inputs(inputs):
    """Host-side shard + compress: returns per-core in_maps."""
    import ml_dtypes
    query = np.asarray(inputs["query"], np.float32)
    mask = np.asarray(inputs["mask"])
    w_q = np.ascontiguousarray(np.asarray(inputs["w_q"], np.float32))
    b_q = np.ascontiguousarray(np.asarray(inputs["b_q"], np.float32))
    wt = np.ascontiguousarray(
        np.asarray(inputs["weight_tensor"], np.float32).reshape(1, H_DIM))

    B, S, _ = query.shape
    f8 = ml_dtypes.float8_e4m3
    in_maps = []
    for b in range(B):
        qT = np.ascontiguousarray(query[b].T.astype(np.float16))
        # additive fp8 mask: 0 where kept, MASK_BIAS where masked out
        mb = np.where(mask[b] == 0, np.float32(MASK_BIAS),
                      np.float32(0.0)).astype(f8)
        in_maps.append(dict(queryT=qT, maskb=mb, w_q=w_q, b_q=b_q,
                            weight_tensor=wt))
    return in_maps, B, S


def kernel(**inputs: np.ndarray) -> np.ndarray:
    in_maps, B, S = prep_inputs(inputs)
    assert B == N_CORES
    nc = build_nc(S)
    res = _run(nc, in_maps)
    return np.stack(
        [res.results[b]["out"].astype(np.float32) for b in range(B)], axis=0)


# revision 12
# speedup vs baseline: 2.1919x; 1.0569x over previous
"""Trainium2 Bass kernel for cosine-similarity multi-head attention.

Math (per batch element b):
    context = query @ w_q.T + b_q                    # [S, 120]
    ctx     = context * weight_tensor                # bcast [1,120]
    ctx_n   = ctx / max(||ctx||_2(axis=-1), 1e-12)   # L2 normalize
    scores  = ctx_n @ ctx_n.T                        # [S, S]
    out     = softmax(where(mask==0, -1e9, scores))  # row softmax

Sharding: data-parallel over batch. 8 batch elements -> 8 NeuronCores.

Structure (v3):
  * Host compresses inputs: mask -> additive fp8 bias {-64, 0} (16 MiB),
    query -> pre-transposed fp16 (4 MiB). Output is fp16 (32 MiB),
    upcast on host. Total DMA ~52 MiB/core vs 136 MiB for the naive
    fp32/int32 layout.
  * Phase 1 builds a normalized-transposed fp16 context ctxT [120, S]:
    ctx tile via PE (fp16 weights -> fast FWL weight loads), row norms
    via ACT Square+Sqrt + DVE reciprocal, scale on DVE, PE transpose.
  * Phase 2 per 128-row tile: PE injects the additive fp8 mask into
    PSUM (identity-weights matmul) then accumulates fp16 cosine scores
    on top; ACT exp (PSUM -> fp16 SBUF) with accum_out row sums; DVE
    reciprocal + in-place 4x-mode normalize; gpsimd SWDGE stores.
  * A 16-matmul warmup burst at the end of phase 1 keeps the PE HAM
    clock-gate at 2.4 GHz going into phase 2.
"""

import sys

if "/opt/trn_rl_repo" not in sys.path:
    sys.path.insert(0, "/opt/trn_rl_repo")

from contextlib import ExitStack

import numpy as np

import concourse.bass as bass
import concourse.mybir as mybir
import concourse.tile as tile
from concourse import bacc
from concourse.masks import make_identity

D_MODEL = 512
H_DIM = 120
N_CORES = 8
P = 128
MASK_BIAS = -64.0  # additive mask value; exp(score + MASK_BIAS) == 0 in fp16

F32 = mybir.dt.float32
F16 = mybir.dt.float16
F8 = mybir.dt.float8e4
Alu = mybir.AluOpType
Act = mybir.ActivationFunctionType

CFG = dict(
    mask_bufs=12,    # fp8 mask tiles [128, S] (4KB/partition each)
    ech_bufs=6,      # fp16 exp/out tiles [128, S] (8KB/partition each)
    qt_bufs=6,       # phase-1 transposed-query tiles (fp16, 1KB each)
    sum_bufs=6,
    ph1_depth=4,     # phase-1 pipeline depth (stage pools)
    newton=False,    # refine ACT sqrt with one Newton step
    warmup0_mms=16,  # PE warmup burst in phase 0 (HAM -> 2.4 GHz early)
    warmup_mms=8,    # insurance burst at phase-1 end
)


def build_nc(S: int = 4096, has_bias: bool = True):
    nc = bacc.Bacc("TRN2", target_bir_lowering=False, debug=False)

    qT_dram = nc.dram_tensor("queryT", [D_MODEL, S], F16, kind="ExternalInput")
    m_dram = nc.dram_tensor("maskb", [S, S], F8, kind="ExternalInput")
    wq_dram = nc.dram_tensor("w_q", [H_DIM, D_MODEL], F32, kind="ExternalInput")
    bq_dram = nc.dram_tensor("b_q", [H_DIM], F32, kind="ExternalInput")
    wt_dram = nc.dram_tensor("weight_tensor", [1, H_DIM], F32, kind="ExternalInput")
    out_dram = nc.dram_tensor("out", [S, S], F16, kind="ExternalOutput")

    NT = S // P          # 128-row tiles
    ND = D_MODEL // P    # 4 contraction chunks
    HCH = 2048           # phase-2 half-chunk (4 PSUM banks)
    NH = S // HCH

    with tile.TileContext(nc) as tc, ExitStack() as ctx:
        singles = ctx.enter_context(tc.tile_pool(name="singles", bufs=1))

        # ---------- Phase 0: constants ----------
        ident = singles.tile([P, P], F32)
        make_identity(nc, ident)
        # fp8 identity: stationary weights for the PE mask injection
        ident8 = singles.tile([P, P], F8)
        nc.vector.tensor_copy(ident8, ident)
        ident16 = singles.tile([P, P], F16)
        nc.vector.tensor_copy(ident16, ident)

        wtb = singles.tile([P, H_DIM], F32)
        nc.gpsimd.dma_start(
            out=wtb,
            in_=bass.AP(tensor=wt_dram, offset=0, ap=[[0, P], [1, H_DIM]]),
        )
        bq_sb = singles.tile([1, H_DIM], F32)
        nc.gpsimd.dma_start(
            out=bq_sb,
            in_=bass.AP(tensor=bq_dram, offset=0, ap=[[0, 1], [1, H_DIM]]),
        )
        bw = singles.tile([1, H_DIM], F32)
        nc.vector.tensor_mul(bw, bq_sb, wtb[:1, :])

        ones_row = singles.tile([1, P], F32)
        nc.vector.memset(ones_row, 1.0)

        # w_q [120, 512] -> transposed+scaled fp16 wqTs [4x128, 120]
        wq_sb = singles.tile([H_DIM, D_MODEL], F32)
        nc.sync.dma_start(out=wq_sb, in_=wq_dram.ap())
        wqTs = singles.tile([P, ND * H_DIM], F16)
        with ExitStack() as ph0:
            ps_w = ph0.enter_context(
                tc.tile_pool(name="ps_w", bufs=2, space="PSUM"))
            for c in range(ND):
                wqT_ps = ps_w.tile([P, H_DIM], F32)
                nc.tensor.transpose(
                    wqT_ps, wq_sb[:, c * P:(c + 1) * P], ident[:H_DIM, :H_DIM])
                nc.vector.tensor_mul(
                    wqTs[:, c * H_DIM:(c + 1) * H_DIM], wqT_ps, wtb)

        # persistent normalized-transposed context [120 (pad 128), S], fp16
        ctxT = singles.tile([P, S], F16)

        # dense warm-up matmuls right away: flips the PE HAM clock-gate to
        # 2.4 GHz before phase 1 so its small matmuls run at full clock
        warm_sb = singles.tile([P, 512], F16)
        nc.vector.memset(warm_sb, 0.5)

        # ---------- Phase 1: build ctxT ----------
        D1 = CFG["ph1_depth"]
        with ExitStack() as ph1:
            qt_p = ph1.enter_context(tc.tile_pool(name="qt", bufs=CFG["qt_bufs"]))
            sq_p = ph1.enter_context(tc.tile_pool(name="sq", bufs=D1))
            st_p = ph1.enter_context(tc.tile_pool(name="ph1st", bufs=D1))
            cn_p = ph1.enter_context(tc.tile_pool(name="ctxn", bufs=D1))
            ps_c = ph1.enter_context(
                tc.tile_pool(name="ps_c", bufs=D1, space="PSUM"))
            ps_ct = ph1.enter_context(
                tc.tile_pool(name="ps_ct", bufs=3, space="PSUM"))
            ps_warm = ph1.enter_context(
                tc.tile_pool(name="ps_warm", bufs=1, space="PSUM"))

            warm0 = ps_warm.tile([P, 512], F32, tag="w0")
            for w in range(CFG["warmup0_mms"]):
                nc.tensor.matmul(
                    warm0, lhsT=warm_sb[:, :P], rhs=warm_sb,
                    start=True, stop=True)

            for i in range(NT):
                s0 = i * P
                # transposed query tile [d=128 (per chunk), 4 chunks x 128 s]
                qt = qt_p.tile([P, D_MODEL], F16)
                nc.gpsimd.dma_start(
                    out=qt,
                    in_=bass.AP(tensor=qT_dram, offset=s0,
                                ap=[[S, P], [P * S, ND], [1, P]]),
                )

                # ctx tile [s=128, k=120] = q @ (w_q * wt).T + b*wt
                ctx_ps = ps_c.tile([P, H_DIM], F32)
                for c in range(ND):
                    nc.tensor.matmul(
                        ctx_ps,
                        lhsT=qt[:, c * P:(c + 1) * P],
                        rhs=wqTs[:, c * H_DIM:(c + 1) * H_DIM],
                        start=(c == 0),
                        stop=(c == ND - 1 and not has_bias))
                if has_bias:
                    nc.tensor.matmul(
                        ctx_ps, lhsT=ones_row, rhs=bw, start=False, stop=True)

                # row L2 norm^2 via ACT Square with free-dim accumulate
                sq = sq_p.tile([P, H_DIM], F32, tag="sq")
                nsq = st_p.tile([P, 1], F32, tag="nsq")
                nc.scalar.activation(sq, ctx_ps, Act.Square, accum_out=nsq)

                sroot = st_p.tile([P, 1], F32, tag="sroot")
                nc.scalar.activation(sroot, nsq, Act.Sqrt)
                if CFG["newton"]:
                    # rstd = 2 / (s0 + nsq/s0): one Newton step for 1/sqrt
                    r0 = st_p.tile([P, 1], F32, tag="r0")
                    nc.vector.reciprocal(r0, sroot)
                    t1 = st_p.tile([P, 1], F32, tag="t1")
                    nc.vector.tensor_mul(t1, nsq, r0)
                    ssum = st_p.tile([P, 1], F32, tag="ssum")
                    nc.vector.tensor_add(ssum, sroot, t1)
                    half = st_p.tile([P, 1], F32, tag="half")
                    nc.vector.tensor_scalar(half, ssum, 0.5, None, Alu.mult)
                    rstd = st_p.tile([P, 1], F32, tag="rstd")
                    nc.vector.reciprocal(rstd, half)
                else:
                    rstd = st_p.tile([P, 1], F32, tag="rstd")
                    nc.vector.reciprocal(rstd, sroot)

                # normalize (fp16) + transpose into ctxT
                ctxn = cn_p.tile([P, H_DIM], F16, tag="ctxn")
                nc.vector.tensor_scalar_mul(ctxn, ctx_ps, rstd)
                ctxT_ps = ps_ct.tile([H_DIM, P], F16)
                nc.tensor.transpose(ctxT_ps, ctxn, ident16)
                if i % 2 == 0:
                    nc.vector.tensor_copy(ctxT[:H_DIM, s0:s0 + P], ctxT_ps)
                else:
                    nc.scalar.copy(ctxT[:H_DIM, s0:s0 + P], ctxT_ps)

            # PE warmup burst: dense matmuls so the HAM clock-gate sits at
            # 2.4 GHz when phase 2 starts. Depends only on ctxT[:, :512].
            warm = ps_warm.tile([P, 512], F32, tag="w0")
            for w in range(CFG["warmup_mms"]):
                nc.tensor.matmul(
                    warm, lhsT=ctxT[:H_DIM, 0:P], rhs=ctxT[:H_DIM, 0:512],
                    start=True, stop=True)

        # ---------- Phase 2: scores + masked softmax ----------
        with ExitStack() as ph2:
            mask_p = ph2.enter_context(
                tc.tile_pool(name="maskp", bufs=CFG["mask_bufs"]))
            ech_p = ph2.enter_context(
                tc.tile_pool(name="echp", bufs=CFG["ech_bufs"]))
            sum_p = ph2.enter_context(
                tc.tile_pool(name="sump", bufs=CFG["sum_bufs"]))
            ps2 = ph2.enter_context(
                tc.tile_pool(name="ps2", bufs=2, space="PSUM"))

            for i in range(NT):
                q0 = i * P
                mask_sb = mask_p.tile([P, S], F8)
                nc.sync.dma_start(out=mask_sb, in_=m_dram[q0:q0 + P, :])

                ech = ech_p.tile([P, S], F16)
                sums = sum_p.tile([P, NH], F32, tag="sums")
                lhsT = ctxT[:H_DIM, q0:q0 + P]
                for h in range(NH):
                    c0 = h * HCH
                    sc = ps2.tile([P, HCH], F32)
                    # PE injects the additive fp8 mask (identity weights),
                    # then accumulates the cosine scores on top.
                    for j in range(HCH // 512):
                        nc.tensor.matmul(
                            sc[:, j * 512:(j + 1) * 512],
                            lhsT=ident8,
                            rhs=mask_sb[:, c0 + j * 512:c0 + (j + 1) * 512],
                            start=True, stop=False)
                    for j in range(HCH // 512):
                        nc.tensor.matmul(
                            sc[:, j * 512:(j + 1) * 512],
                            lhsT=lhsT,
                            rhs=ctxT[:H_DIM, c0 + j * 512:c0 + (j + 1) * 512],
                            start=False, stop=True)
                    # exp -> fp16 (masked entries flush to exactly 0);
                    # accum_out gives the fp32 row sum for free
                    nc.scalar.activation(
                        ech[:, c0:c0 + HCH], sc, Act.Exp,
                        accum_out=sums[:, h:h + 1])

                s2 = sum_p.tile([P, 1], F32, tag="s2")
                nc.vector.tensor_add(s2, sums[:, 0:1], sums[:, 1:2])
                rden = sum_p.tile([P, 1], F32, tag="rden")
                nc.vector.reciprocal(rden, s2)

                # normalize in place (fp16 4x mode), store via gpsimd SWDGE
                for h in range(NH):
                    c0 = h * HCH
                    nc.vector.tensor_scalar_mul(
                        ech[:, c0:c0 + HCH], ech[:, c0:c0 + HCH], rden)
                    nc.gpsimd.dma_start(
                        out=out_dram[q0:q0 + P, c0:c0 + HCH],
                        in_=ech[:, c0:c0 + HCH])

    nc.compile()
    return nc


def _run(nc, in_maps, trace=False, tmpdir=None):
    from concourse import bass_utils
    return bass_utils.run_bass_kernel_spmd(
        nc, in_maps, core_ids=list(range(len(in_maps))), trace=trace,
        tmpdir=tmpdir)


def prep_inputs(inputs):
    """Host-side shard + compress: returns per-core in_maps."""
    import ml_dtypes
    query = np.asarray(inputs["query"], np.float32)
    mask = np.asarray(inputs["mask"])
    w_q = np.ascontiguousarray(np.asarray(inputs["w_q"], np.float32))
    b_q = np.ascontiguousarray(np.asarray(inputs["b_q"], np.float32))
    wt = np.ascontiguousarray(
        np.asarray(inputs["weight_tensor"], np.float32).reshape(1, H_DIM))

    B, S, _ = query.shape
    f8 = ml_dtypes.float8_e4m3
    in_maps = []
    for b in range(B):
        qT = np.ascontiguousarray(query[b].T.astype(np.float16))
        # additive fp8 mask: 0 where kept, MASK_BIAS where masked out
        mb = np.where(mask[b] == 0, np.float32(MASK_BIAS),
                      np.float32(0.0)).astype(f8)
        in_maps.append(dict(queryT=qT, maskb=mb, w_q=w_q, b_q=b_q,
                            weight_tensor=wt))
    has_bias = bool(np.any(b_q != 0))
    return in_maps, B, S, has_bias


def kernel(**inputs: np.ndarray) -> np.ndarray:
    in_maps, B, S, has_bias = prep_inputs(inputs)
    assert B == N_CORES
    nc = build_nc(S, has_bias=has_bias)
    res = _run(nc, in_maps)
    return np.stack(
        [res.results[b]["out"].astype(np.float32) for b in range(B)], axis=0)
